# revision 22
# baseline (speedup 1.0000x reference)
"""Trainium2 Bass kernel for PointSetAttention (GNN edge-softmax attention).

Strategy (8 NeuronCores, SPMD):
  - Nodes are partitioned into 8 contiguous ranges by destination id; edges are
    owned by the core that owns their dst node, so the segment softmax/sum
    never crosses cores (no collectives needed).
  - The reference's rpe logit collapses per edge to a[dst,h] - a[src,h] + b_h
    with a[n,h] = coord[n] . (sum_d rpe_w[h*32+d]), b_h = sum_d rpe_b[h*32+d],
    because rpe is summed over D. The a[dst] term is constant per softmax
    segment and cancels; the remaining per-src term factors out of the exp:
        softmax-weighted sum = sum_e exp(qk_e) * w_src * v_src
                             / sum_e exp(qk_e) * w_src,
    with w[n,h] = exp(b_h - a[n,h]) a per-node factor.
  - Phase 1 (device, node-sharded): qkv projection; w = exp(b - a);
    wv = w * v. Host then assembles gather tables (pure relayout):
        Q table row  [NPAD, 128] f32: q (4 heads x 32)
        KV table row [KVN, 320] f32: per head 80: [k_h(32), wv_h(32), w_h, 0..]
  - Phase 2 (device): per dst-node block of 128 nodes, dma_gather the KV rows
    of its edges (edge chunks of 128 on partitions, groups of 4 chunks),
    expand per-edge q rows via a one-hot transposed-mask matmul on PE
    (q is never gathered from HBM), per-edge logits via one batched DVE
    multiply + strided reduce per group, exp on ACT, then segment-sum via a
    one-hot mask matmul accumulated in PSUM:
      psum[n, (h,j)] += sum_e mask[e,n] * (ex[e,h] * [wv_h(32), w_h][j])
    out = psum[:, :32-block] * reciprocal(trailing column).
  - No max-subtraction is needed: logits stay within +-~45 so exp stays in
    f32 range, and all segment terms are positive (no cancellation).
  - int16 gather indices can't span 50000 rows, so src gathers are split into
    a low table view (rows [0, 32768)) and a high view whose base is chosen so
    the largest index is 32767, with each block's edges grouped lo-then-hi
    (padded to chunk granularity; pads gather row 0 and are masked out via
    dst_local = -1 -> all-zero mask column).
"""

import sys

sys.path.insert(0, "/opt/trn_rl_repo")

import numpy as np

from concourse import bacc, mybir, tile
from concourse.bass_utils import run_bass_kernel_spmd

F32 = mybir.dt.float32
BF16 = mybir.dt.bfloat16
I16 = mybir.dt.int16

C = 128
H = 4
D = 32
POS = 3
BLK = 128
GCALL = 1024             # max rows per dma_gather call (SWDGE ring capacity)
GRP = 8                  # chunks per compute group (8*128 f32 = two PSUM banks)


def _set_dims(n, e, ncores, lo=32768):
    global N, E, NC, NPC, NBLK, NPAD, KVN, LO, HIB
    N, E, NC = n, e, ncores
    NPC = N // NC
    NBLK = (NPC + BLK - 1) // BLK
    NPAD = NBLK * BLK
    KVN = ((N + 127) // 128) * 128 + 48
    LO = lo
    HIB = max(N - lo, 0)     # hi-table base: max idx = N-1-HIB <= lo-1 <= 32767


_set_dims(50000, 800000, 8)

QROW = 128               # f16 per Q table row (plain q, 4 heads x 32)
KVROW = 384              # bf16 units per KV row: [k f16(128) | (wv,w) bf16(132) | pad]
F16 = mybir.dt.float16


def _wrap_cols(idx2d):
    """[G, num] int -> [128, G*num//16] int16 in dma_gather's wrapped layout.

    Index j of group g lands at [j % 16, g*(num//16) + j//16], replicated
    across the 8 16-partition groups.
    """
    G, num = idx2d.shape
    cols = num // 16
    # element (p, g*cols + s) = idx2d[g, s*16 + p]
    w = idx2d.reshape(G, cols, 16).transpose(2, 0, 1).reshape(16, G * cols)
    return np.tile(w.astype(np.int16), (8, 1))


def _plan(graph):
    """Host-side edge plan: per-(core, block) lo/hi src lists + dst arrays."""
    dst = np.asarray(graph[0], dtype=np.int64)
    src = np.asarray(graph[1], dtype=np.int64)
    core = dst // NPC
    lb = (dst - core * NPC) // BLK
    key = core * NBLK + lb
    hi = (src >= LO).astype(np.int64)
    order = np.lexsort((src, hi, key))
    dsts, srcs, keys, his = dst[order], src[order], key[order], hi[order]

    ngrp = NC * NBLK
    cnt_lo = np.bincount(keys[his == 0], minlength=ngrp)
    cnt_hi = np.bincount(keys[his == 1], minlength=ngrp)
    clo = max(int((cnt_lo.max() + BLK - 1) // BLK), 1)
    chi = max(int((cnt_hi.max() + BLK - 1) // BLK), 1)
    cpb = clo + chi

    gid = keys * 2 + his
    start = np.zeros(2 * ngrp + 1, dtype=np.int64)
    np.add.at(start, gid + 1, 1)
    start = np.cumsum(start)
    pos_in_grp = np.arange(E) - start[gid]
    slot = np.where(his == 0, pos_in_grp, clo * BLK + pos_in_grp)
    gslot = keys * (cpb * BLK) + slot

    kvidx = np.zeros(ngrp * cpb * BLK, dtype=np.int64)
    dstcol = np.full(ngrp * cpb * BLK, -1.0, dtype=np.float32)
    kvidx[gslot] = np.where(his == 0, srcs, srcs - HIB)
    dstcol[gslot] = (dsts - (keys // NBLK) * NPC - (keys % NBLK) * BLK).astype(
        np.float32
    )

    kvidx = kvidx.reshape(NC, NBLK, cpb * BLK)
    dstcol = dstcol.reshape(NC, NBLK, cpb, BLK)

    plans = []
    ar = np.arange(BLK, dtype=np.float32)
    for c in range(NC):
        lo_w = _wrap_cols(kvidx[c, :, : clo * BLK])          # [128, NBLK*clo*8]
        hi_w = _wrap_cols(kvidx[c, :, clo * BLK :])          # [128, NBLK*chi*8]
        # host-built one-hot scatter masks, bf16: [128e, NBLK*cpb*128n]
        # edge p of chunk t of block b -> column range (b*cpb+t)*128
        import ml_dtypes

        m = dstcol[c][:, :, :, None] == ar[None, None, None, :]
        mT = m.transpose(3, 0, 1, 2).reshape(BLK, NBLK * cpb * BLK)
        m = m.transpose(2, 0, 1, 3).reshape(BLK, NBLK * cpb * BLK)
        masks = np.ascontiguousarray(m.astype(ml_dtypes.bfloat16))
        masksT = np.ascontiguousarray(mT.astype(np.float16))
        plans.append({"lo": lo_w, "hi": hi_w, "mask": masks, "maskT": masksT})
    return plans, clo, chi, cpb


def _build_phase1():
    nc = bacc.Bacc("TRN2", target_bir_lowering=False)
    featT = nc.dram_tensor("featT", [C, NPAD], F32, kind="ExternalInput")
    wt = nc.dram_tensor("wt", [C, 3 * C], F32, kind="ExternalInput")
    bias = nc.dram_tensor("bias", [C, 3], F32, kind="ExternalInput")
    coordT = nc.dram_tensor("coordT", [POS, NPAD], F32, kind="ExternalInput")
    wsT = nc.dram_tensor("wsT", [POS, H], F32, kind="ExternalInput")
    bsum = nc.dram_tensor("bsum", [H, 1], F32, kind="ExternalInput")
    hrep = nc.dram_tensor("hrep", [H, C], F32, kind="ExternalInput")
    qkT = nc.dram_tensor("qkT", [2 * C, NPAD], F32, kind="ExternalOutput")
    wvT = nc.dram_tensor("wvT", [C, NPAD], F32, kind="ExternalOutput")
    wT = nc.dram_tensor("wT", [H, NPAD], F32, kind="ExternalOutput")

    CH = 512
    nch = (NPAD + CH - 1) // CH
    with tile.TileContext(nc) as tc:
        with (
            tc.tile_pool(name="cst", bufs=1) as cst,
            tc.tile_pool(name="sb", bufs=3) as sb,
            tc.tile_pool(name="ps", bufs=2, space="PSUM") as ps,
        ):
            f_sb = cst.tile([C, NPAD], F32)
            nc.sync.dma_start(out=f_sb[:], in_=featT[:])
            w_sb = cst.tile([C, 3 * C], F32)
            nc.sync.dma_start(out=w_sb[:], in_=wt[:])
            b_sb = cst.tile([C, 3], F32)
            nc.sync.dma_start(out=b_sb[:], in_=bias[:])
            c_sb = cst.tile([POS, NPAD], F32)
            nc.sync.dma_start(out=c_sb[:], in_=coordT[:])
            ws_sb = cst.tile([POS, H], F32)
            nc.sync.dma_start(out=ws_sb[:], in_=wsT[:])
            bs_sb = cst.tile([H, 1], F32)
            nc.sync.dma_start(out=bs_sb[:], in_=bsum[:])

            hrep_sb = cst.tile([H, C], F32)
            nc.sync.dma_start(out=hrep_sb[:], in_=hrep[:])

            for m in range(3):
                for j in range(nch):
                    w = min(CH, NPAD - j * CH)
                    p = ps.tile([C, CH], F32, tag="p1")
                    nc.tensor.matmul(
                        out=p[:, :w],
                        lhsT=w_sb[:, m * C : (m + 1) * C],
                        rhs=f_sb[:, j * CH : j * CH + w],
                        start=True,
                        stop=True,
                    )
                    o = sb.tile([C, CH], F32, tag="o1")
                    nc.vector.tensor_scalar(
                        out=o[:, :w],
                        in0=p[:, :w],
                        scalar1=b_sb[:, m : m + 1],
                        scalar2=None,
                        op0=mybir.AluOpType.add,
                    )
                    if m < 2:
                        nc.sync.dma_start(
                            out=qkT[m * C : (m + 1) * C, j * CH : j * CH + w],
                            in_=o[:, :w],
                        )
                    else:
                        # w[h, n] = exp(bsum_h - a[h, n]), a = ws @ coord
                        pa = ps.tile([H, CH], F32, tag="pa")
                        nc.tensor.matmul(
                            out=pa[:, :w],
                            lhsT=ws_sb[:],
                            rhs=c_sb[:, j * CH : j * CH + w],
                            start=True,
                            stop=True,
                        )
                        neg = sb.tile([H, CH], F32, tag="neg")
                        nc.vector.tensor_scalar(
                            out=neg[:, :w],
                            in0=pa[:, :w],
                            scalar1=-1.0,
                            scalar2=bs_sb[:, 0:1],
                            op0=mybir.AluOpType.mult,
                            op1=mybir.AluOpType.add,
                        )
                        wrow = sb.tile([H, CH], F32, tag="wrow")
                        nc.scalar.activation(
                            out=wrow[:, :w], in_=neg[:, :w],
                            func=mybir.ActivationFunctionType.Exp,
                        )
                        nc.sync.dma_start(
                            out=wT[:, j * CH : j * CH + w], in_=wrow[:, :w]
                        )
                        # replicate each head's w row across its 32 channels
                        wrep = ps.tile([C, CH], F32, tag="wrep")
                        nc.tensor.matmul(
                            out=wrep[:, :w],
                            lhsT=hrep_sb[:],
                            rhs=wrow[:, :w],
                            start=True,
                            stop=True,
                        )
                        wv = sb.tile([C, CH], F32, tag="wv")
                        nc.vector.tensor_tensor(
                            out=wv[:, :w],
                            in0=o[:, :w],
                            in1=wrep[:, :w],
                            op=mybir.AluOpType.mult,
                        )
                        nc.sync.dma_start(
                            out=wvT[:, j * CH : j * CH + w], in_=wv[:, :w]
                        )
    nc.finalize()
    return nc


def _gcalls(nchunks):
    out = []
    t = 0
    per = GCALL // BLK
    while t < nchunks:
        n = min(per, nchunks - t)
        out.append((t, n))
        t += n
    return out


def _groups(cpb):
    out = []
    t = 0
    while t < cpb:
        g = min(GRP, cpb - t)
        out.append((t, g))
        t += g
    return out


def _build_phase2(clo, chi, locols, hicols):
    cpb = clo + chi
    nc = bacc.Bacc("TRN2", target_bir_lowering=False)
    qtab = nc.dram_tensor("qtab", [NPAD, QROW], F16, kind="ExternalInput")
    kvtab = nc.dram_tensor("kvtab", [KVN, KVROW], BF16, kind="ExternalInput")
    loidx = nc.dram_tensor("loidx", [128, locols], I16, kind="ExternalInput")
    hiidx = nc.dram_tensor("hiidx", [128, hicols], I16, kind="ExternalInput")
    maskd = nc.dram_tensor(
        "maskd", [128, NBLK * cpb * BLK], BF16, kind="ExternalInput"
    )
    masktd = nc.dram_tensor(
        "masktd", [128, NBLK * cpb * BLK], F16, kind="ExternalInput"
    )
    outd = nc.dram_tensor("out", [NPAD, C], F32, kind="ExternalOutput")

    MB = 4                      # blocks per mask load batch
    lc8, hc8 = clo * 8, chi * 8
    with tile.TileContext(nc) as tc:
        with (
            tc.tile_pool(name="cst", bufs=1) as cst,
            tc.tile_pool(name="gkv", bufs=2) as gkv,
            tc.tile_pool(name="msk", bufs=2) as msk,
            tc.tile_pool(name="sm", bufs=2) as sm,
            tc.tile_pool(name="ob", bufs=2) as obp,
            tc.tile_pool(name="ps", bufs=2, space="PSUM") as ps,
            tc.tile_pool(name="psq", bufs=2, space="PSUM") as psq,
        ):
            lo_sb = cst.tile([128, locols], I16)
            nc.sync.dma_start(out=lo_sb[:], in_=loidx[:])
            hi_sb = cst.tile([128, hicols], I16)
            nc.sync.dma_start(out=hi_sb[:], in_=hiidx[:])
            # all q rows resident: qall[p, b, ch] = qtab[b*128+p, ch]
            qall = cst.tile([128, NBLK, QROW], F16)
            nc.sync.dma_start(
                out=qall[:],
                in_=qtab[:].rearrange("(b p) c -> p b c", p=BLK),
            )

            for b in range(NBLK):
                mb, mo = b // MB, b % MB
                nmb = min(MB, NBLK - mb * MB)
                if mo == 0:
                    mk_sb = msk.tile([128, MB * cpb * BLK], BF16, tag="mk")
                    nc.sync.dma_start(
                        out=mk_sb[:, : nmb * cpb * BLK],
                        in_=maskd[
                            :,
                            mb * MB * cpb * BLK : (mb * MB + nmb) * cpb * BLK,
                        ],
                    )
                    mt_sb = msk.tile([128, MB * cpb * BLK], F16, tag="mt")
                    nc.sync.dma_start(
                        out=mt_sb[:, : nmb * cpb * BLK],
                        in_=masktd[
                            :,
                            mb * MB * cpb * BLK : (mb * MB + nmb) * cpb * BLK,
                        ],
                    )
                kvg = gkv.tile([128, cpb, KVROW], BF16)
                for t0, nt in _gcalls(clo):
                    nc.gpsimd.dma_gather(
                        out_ap=kvg[:, t0 : t0 + nt, :],
                        in_ap=kvtab[0:LO, :],
                        idxs_ap=lo_sb[:, b * lc8 + t0 * 8 : b * lc8 + (t0 + nt) * 8],
                        num_idxs=nt * BLK,
                        num_idxs_reg=nt * BLK,
                        elem_size=KVROW,
                    )
                for t0, nt in _gcalls(chi):
                    nc.gpsimd.dma_gather(
                        out_ap=kvg[:, clo + t0 : clo + t0 + nt, :],
                        in_ap=kvtab[HIB:KVN, :],
                        idxs_ap=hi_sb[:, b * hc8 + t0 * 8 : b * hc8 + (t0 + nt) * 8],
                        num_idxs=nt * BLK,
                        num_idxs_reg=nt * BLK,
                        elem_size=KVROW,
                    )

                psum = ps.tile([128, H * (D + 1)], F32)
                for g0, gn in _groups(cpb):
                    # expand q rows per chunk on PE: qx[128e, 128ch] slices
                    qx = psq.tile([128, GRP * QROW], F32, tag="qx")
                    for g in range(gn):
                        t = g0 + g
                        nc.tensor.matmul(
                            out=qx[:, g * QROW : (g + 1) * QROW],
                            lhsT=mt_sb[
                                :,
                                (mo * cpb + t) * BLK : (mo * cpb + t + 1) * BLK,
                            ],
                            rhs=qall[:, b, :],
                            start=True,
                            stop=True,
                        )
                    # per-edge q.k products and logits
                    prod = sm.tile([128, GRP * QROW], F16, tag="prod")
                    nc.vector.tensor_tensor(
                        out=prod[:, : gn * QROW].rearrange(
                            "p (g h c) -> p g h c", h=H, c=D
                        ),
                        in0=qx[:, : gn * QROW].rearrange(
                            "p (g h c) -> p g h c", h=H, c=D
                        ),
                        in1=kvg[:, g0 : g0 + gn, 0:QROW]
                        .bitcast(F16)
                        .rearrange("p g (h c) -> p g h c", c=D),
                        op=mybir.AluOpType.mult,
                    )
                    logit = sm.tile([128, GRP * H], F32, tag="logit")
                    nc.vector.tensor_reduce(
                        out=logit[:, : gn * H],
                        in_=prod[:, : gn * QROW].rearrange(
                            "p (g h c) -> p g h c", h=H, c=D
                        ),
                        axis=mybir.AxisListType.X,
                        op=mybir.AluOpType.add,
                    )
                    ex = sm.tile([128, GRP * H], F32, tag="ex")
                    nc.scalar.activation(
                        out=ex[:, : gn * H], in_=logit[:, : gn * H],
                        func=mybir.ActivationFunctionType.Exp,
                    )
                    # rhs = ex * [wv_h(32), w_h]
                    rhs = sm.tile([128, GRP, H, D + 1], BF16, tag="rhs")
                    nc.vector.tensor_tensor(
                        out=rhs[:, :gn, :, :],
                        in0=ex[:, : gn * H]
                        .rearrange("p (g h) -> p g h", h=H)
                        .unsqueeze(3)
                        .to_broadcast([128, gn, H, D + 1]),
                        in1=kvg[:, g0 : g0 + gn, QROW : QROW + H * (D + 1)]
                        .rearrange("p g (h c) -> p g h c", c=D + 1),
                        op=mybir.AluOpType.mult,
                    )
                    for g in range(gn):
                        t = g0 + g
                        nc.tensor.matmul(
                            out=psum[:],
                            lhsT=mk_sb[
                                :,
                                (mo * cpb + t) * BLK : (mo * cpb + t + 1) * BLK,
                            ],
                            rhs=rhs[:, g, :, :].rearrange("p h c -> p (h c)"),
                            start=(t == 0),
                            stop=(t == cpb - 1),
                        )
                pv = psum[:].rearrange("p (h c) -> p h c", c=D + 1)
                sums = obp.tile([128, H], F32, tag="sums")
                nc.vector.tensor_scalar(
                    out=sums[:].rearrange("p (h o) -> p h o", o=1),
                    in0=pv[:, :, D : D + 1],
                    scalar1=1e-38,
                    scalar2=None,
                    op0=mybir.AluOpType.add,
                )
                rec = obp.tile([128, H], F32, tag="rec")
                nc.vector.reciprocal(rec[:], sums[:])
                outb = obp.tile([128, C], F32, tag="outb")
                nc.vector.tensor_tensor(
                    out=outb[:].rearrange("p (h c) -> p h c", c=D),
                    in0=pv[:, :, 0:D],
                    in1=rec[:]
                    .rearrange("p (h o) -> p h o", o=1)
                    .to_broadcast([128, H, D]),
                    op=mybir.AluOpType.mult,
                )
                nc.sync.dma_start(
                    out=outd[b * BLK : (b + 1) * BLK, :], in_=outb[:]
                )
    nc.finalize()
    return nc


def _prep_phase1_inputs(feat, coord, qkv_w, qkv_b, rpe_w, rpe_b):
    featT = np.ascontiguousarray(feat.T)
    coordT = np.ascontiguousarray(coord.T)
    wt = np.ascontiguousarray(qkv_w.T)
    bias = np.ascontiguousarray(qkv_b.reshape(3, C).T)
    ws = rpe_w.reshape(H, D, POS).sum(axis=1)
    wsT = np.ascontiguousarray(ws.T)
    bsum = rpe_b.reshape(H, D).sum(axis=1).reshape(H, 1).astype(np.float32)
    hrep = np.zeros((H, C), dtype=np.float32)
    for h in range(H):
        hrep[h, h * D : (h + 1) * D] = 1.0
    in1 = []
    for c in range(NC):
        fT = np.zeros((C, NPAD), dtype=np.float32)
        fT[:, :NPC] = featT[:, c * NPC : (c + 1) * NPC]
        cT = np.zeros((POS, NPAD), dtype=np.float32)
        cT[:, :NPC] = coordT[:, c * NPC : (c + 1) * NPC]
        in1.append(
            {"featT": fT, "wt": wt, "bias": bias, "coordT": cT, "wsT": wsT,
             "bsum": bsum, "hrep": hrep}
        )
    return in1


def _assemble_tables(r1_results):
    import ml_dtypes

    kT = np.concatenate(
        [r1_results[c]["qkT"][C : 2 * C, :NPC] for c in range(NC)], axis=1
    )
    wvT = np.concatenate(
        [r1_results[c]["wvT"][:, :NPC] for c in range(NC)], axis=1
    )
    wT = np.concatenate([r1_results[c]["wT"][:, :NPC] for c in range(NC)], axis=1)
    # KV row (bf16 units): [k f16 (128) | per head (wv(32), w) bf16 (132) | pad]
    kvtab = np.zeros((KVN, KVROW), dtype=ml_dtypes.bfloat16)
    kvtab[:N, 0:C] = kT.T.astype(np.float16).view(ml_dtypes.bfloat16)
    wvw = np.zeros((N, H, D + 1), dtype=ml_dtypes.bfloat16)
    for h in range(H):
        wvw[:, h, 0:D] = wvT[h * D : (h + 1) * D, :].T.astype(ml_dtypes.bfloat16)
        wvw[:, h, D] = wT[h, :].astype(ml_dtypes.bfloat16)
    kvtab[:N, C : C + H * (D + 1)] = wvw.reshape(N, H * (D + 1))

    qtabs = []
    for c in range(NC):
        qt = np.ascontiguousarray(
            r1_results[c]["qkT"][0:C, :].T.astype(np.float16)
        )
        qtabs.append(qt)
    return kvtab, qtabs


def _prep_phase2_inputs(plans, qtabs, kvtab):
    return [
        {
            "qtab": qtabs[c],
            "kvtab": kvtab,
            "loidx": plans[c]["lo"],
            "hiidx": plans[c]["hi"],
            "maskd": plans[c]["mask"],
            "masktd": plans[c]["maskT"],
        }
        for c in range(NC)
    ]


def kernel(feat, coord, graph, qkv_w, qkv_b, rpe_w, rpe_b):
    feat = np.asarray(feat, dtype=np.float32)
    coord = np.asarray(coord, dtype=np.float32)
    graph = np.asarray(graph)
    qkv_w = np.asarray(qkv_w, dtype=np.float32)
    qkv_b = np.asarray(qkv_b, dtype=np.float32)
    rpe_w = np.asarray(rpe_w, dtype=np.float32)
    rpe_b = np.asarray(rpe_b, dtype=np.float32)

    plans, clo, chi, cpb = _plan(graph)

    nc1 = _build_phase1()
    in1 = _prep_phase1_inputs(feat, coord, qkv_w, qkv_b, rpe_w, rpe_b)
    r1 = run_bass_kernel_spmd(nc1, in1, list(range(NC)))

    kvtab, qtabs = _assemble_tables(r1.results)

    p0 = plans[0]
    nc2 = _build_phase2(clo, chi, p0["lo"].shape[1], p0["hi"].shape[1])
    in2 = _prep_phase2_inputs(plans, qtabs, kvtab)
    r2 = run_bass_kernel_spmd(nc2, in2, list(range(NC)))

    out = np.concatenate(
        [r2.results[c]["out"][:NPC, :] for c in range(NC)], axis=0
    ).astype(np.float32)
    return out
